# revision 15
# baseline (speedup 1.0000x reference)
"""Causal multi-head self-attention on 8 Trainium2 NeuronCores.

Sharding: tensor-parallel over heads. 16 heads / 8 cores = 2 heads per core.
Each core gets the W_qkv rows and W_out columns for its 2 heads, the full
(pre-transposed) X, computes its heads' attention plus its slice of the output
projection, and returns a partial [B, S, D] output. The host sums the 8
partials (the "all-reduce" of the TP output projection).

On-device layout strategy (everything stays transposed until the end):
  - qkvT = W_shard @ X^T computed as matmul(lhsT=W^T tile, rhs=X^T tile)
    -> Q^T/K^T/V^T tiles [dv-part, seq-free]; head0 on partitions 0-63,
    head1 on 64-127.
  - scoresT[kpos, q] = matmul(lhsT=K^T tile, rhs=Q^T tile); the two heads run
    concurrently on the PE array via row-tiling (contraction dv=64 each).
  - causal handling: only q >= kpos tiles/columns are computed (partial-width
    matmuls); the 128-wide diagonal block gets a precomputed triangular
    -1e30 mask added before exp.
  - softmax without max-subtraction (scores ~ N(0,1): exp is safe in fp32);
    exp on the scalar engine reads PSUM directly, one call for both heads.
  - V is re-transposed to natural layout with PE transposes; an extra
    all-ones column is appended so the attn@V matmul also produces the
    softmax denominators in PSUM row 64 for free.
  - normalization: reciprocal of the sums row, partition-broadcast via DMA,
    one tensor-tensor multiply into the out^T buffer.
  - output projection y[s,dm] = matmul(lhsT=outT tile [e=128, s], rhs=W_out^T
    shard) with a single k=128 contraction (head1's outT is shifted to
    partitions 64-127 by an SBUF->SBUF DMA).
"""

import numpy as np

import concourse.bacc as bacc
import concourse.bass as bass
import concourse.mybir as mybir
import concourse.tile as tile

FP32 = mybir.dt.float32

B = 4
S = 2048
D = 1024
H = 16
DV = 64
N_CORES = 8
HEADS_PER_CORE = H // N_CORES          # 2
E = HEADS_PER_CORE * DV                # 128 rows of Q/K/V per core
NEG = -1.0e30

# PE matmul operand dtype: float32 is exact but runs at 4 cycles/row on the
# PE; float32r is the single-pass reduced-precision fp32 mode (1 cycle/row
# when N>=256). Storage stays fp32; the switch just reinterprets matmul
# operand APs.
MM_DT = mybir.dt.float32

SQ = 512            # q tile width (PSUM bank)
SK = 128            # kpos tile width (contraction)
N_SQ = S // SQ      # 4 q-tiles per (b, h)
N_SK = S // SK      # 16 kpos tiles
N_D = D // 128      # 8 contraction tiles for the projections
VBLK = 2 * (DV + 1)  # 130: [V_h0 | 1 | V_h1 | 1] per kpos tile


def build_nc() -> bass.Bass:
    # Bacc (not plain Bass): its compile() pass splits multi-wait
    # instructions that walrus codegen otherwise rejects ("Too many sync
    # wait commands" — the ISA has one wait slot per instruction).
    nc = bacc.Bacc(None, target_bir_lowering=False)

    xt = nc.declare_dram_parameter("xt", [B, D, S], FP32, isOutput=False)
    wqT = nc.declare_dram_parameter("wqT", [D, E], FP32, isOutput=False)
    wkT = nc.declare_dram_parameter("wkT", [D, E], FP32, isOutput=False)
    wvT = nc.declare_dram_parameter("wvT", [D, E], FP32, isOutput=False)
    woutT = nc.declare_dram_parameter("woutT", [E, D], FP32, isOutput=False)
    y = nc.declare_dram_parameter("y", [B, S, D], FP32, isOutput=True)

    with tile.TileContext(nc) as tc:
        _build(tc, xt, wqT, wkT, wvT, woutT, y)
    nc.compile()
    return nc


def _build(tc, xt, wqT, wkT, wvT, woutT, y):
    nc = tc.nc

    def mm(out, lhsT, rhs, start, stop):
        if MM_DT != FP32:
            lhsT = lhsT.bitcast(MM_DT)
            rhs = rhs.bitcast(MM_DT)
        nc.tensor.matmul(out, lhsT=lhsT, rhs=rhs, start=start, stop=stop)

    with (
        tc.tile_pool(name="consts", bufs=1) as consts,
        tc.tile_pool(name="xtp", bufs=1) as xtp,
        tc.tile_pool(name="qkp", bufs=2) as qkp,
        tc.tile_pool(name="vtp", bufs=1) as vtp,
        tc.tile_pool(name="vnp", bufs=2) as vnp,
        tc.tile_pool(name="attnp", bufs=3) as attnp,
        tc.tile_pool(name="outp", bufs=2) as outp,
        tc.tile_pool(name="smallp", bufs=2) as smallp,
        tc.tile_pool(name="ystp", bufs=2) as ystp,
        tc.tile_pool(name="dramp", bufs=4, space="DRAM") as dramp,
        tc.tile_pool(name="ps_work", bufs=2, space="PSUM") as ps_work,
        tc.tile_pool(name="ps_scores", bufs=2, space="PSUM") as ps_scores,
        tc.tile_pool(name="ps_av", bufs=2, space="PSUM") as ps_av,
    ):
        # ---- constants ----
        identity = consts.tile([128, 128], FP32)
        nc.gpsimd.memset(identity, 0.0)
        nc.gpsimd.affine_select(
            out=identity, in_=identity,
            compare_op=mybir.AluOpType.not_equal,
            fill=1.0, base=0, pattern=[[-1, 128]], channel_multiplier=1,
        )
        # mask_tri[p, c] = 0 if c >= p else NEG   (valid where q-col >= kpos-row)
        mask_tri = consts.tile([128, 128], FP32)
        nc.gpsimd.memset(mask_tri, 0.0)
        nc.gpsimd.affine_select(
            out=mask_tri, in_=mask_tri,
            compare_op=mybir.AluOpType.is_ge,
            fill=NEG, base=0, pattern=[[1, 128]], channel_multiplier=-1,
        )

        # weights: [D, E] -> [128p, N_D, E] tiles (lhsT layout, contraction on
        # partitions)
        w_sb = {}
        for name, w in (("q", wqT), ("k", wkT), ("v", wvT)):
            t = consts.tile([128, N_D, E], FP32, tag=f"w{name}_sb")
            nc.sync.dma_start(out=t, in_=w[:].rearrange("(t p) e -> p t e", p=128))
            w_sb[name] = t
        wout_sb = consts.tile([128, D], FP32)
        nc.sync.dma_start(out=wout_sb, in_=woutT[:])

        for b in range(B):
            # ---- load X^T for this batch ----
            # one tile+DMA per 128-row block so each matmul waits on only one
            # DMA semaphore (walrus rejects >4 sync waits per instruction)
            xt_sb = [
                xtp.tile([128, S], FP32, tag=f"xt{t}", name=f"xt_sb{t}")
                for t in range(N_D)
            ]
            for t in range(N_D):
                nc.sync.dma_start(
                    out=xt_sb[t], in_=xt[b, 128 * t:128 * (t + 1), :]
                )

            # ---- QKV projections (transposed layout) ----
            qT_sb = qkp.tile([128, S], FP32, tag="qT")
            kT_sb = qkp.tile([128, S], FP32, tag="kT")
            vT_sb = vtp.tile([128, S], FP32)
            for wname, dst in (("q", qT_sb), ("k", kT_sb), ("v", vT_sb)):
                for j in range(N_SQ):
                    ps = ps_work.tile([128, SQ], FP32, tag="ps_work")
                    for d in range(N_D):
                        mm(
                            ps,
                            lhsT=w_sb[wname][:, d, :],
                            rhs=xt_sb[d][:, bass.ts(j, SQ)],
                            start=(d == 0),
                            stop=(d == N_D - 1),
                        )
                    nc.vector.tensor_copy(dst[:, bass.ts(j, SQ)], ps)

            # ---- V -> natural layout with ones columns ----
            # v_sb block i: [V_h0(64) | 1 | V_h1(64) | 1]
            v_sb = vnp.tile([128, N_SK, VBLK], FP32)
            ones_ap = bass.AP(
                tensor=v_sb.tensor,
                offset=v_sb.offset + DV,
                ap=[v_sb.ap[0], [VBLK, N_SK], [DV + 1, 2]],
            )
            nc.vector.memset(ones_ap, 1.0)
            for i in range(N_SK):
                tps = ps_work.tile([128, 128], FP32, tag="ps_work")
                nc.tensor.transpose(tps, vT_sb[:, bass.ts(i, SK)], identity)
                nc.vector.tensor_copy(v_sb[:, i, 0:DV], tps[:, 0:DV])
                nc.vector.tensor_copy(v_sb[:, i, DV + 1:DV + 1 + DV], tps[:, DV:128])

            # ---- attention ----
            outT_full = outp.tile([128, S], FP32, tag="outT_full")
            outT_h1 = outp.tile([64, S], FP32, tag="outT_h1", bufs=1)
            for j in range(N_SQ):
                av_ps = [
                    ps_av.tile([DV + 1, SQ], FP32, tag="av_ps", name=f"av_ps{h}")
                    for h in range(2)
                ]
                n_i = 4 * j + 4
                for i in range(n_i):
                    s0 = max(0, SK * i - SQ * j)  # first valid col in q block
                    w = SQ - s0
                    sc_ps = ps_scores.tile([128, 2 * SQ], FP32, tag="sc_ps")
                    for h in range(2):
                        mm(
                            sc_ps[:, SQ * h + s0:SQ * (h + 1)],
                            lhsT=kT_sb[DV * h:DV * (h + 1), bass.ts(i, SK)],
                            rhs=qT_sb[DV * h:DV * (h + 1), SQ * j + s0:SQ * (j + 1)],
                            start=True,
                            stop=True,
                        )
                    if i >= 4 * j:  # diagonal tile: triangular mask on 128 cols
                        for h in range(2):
                            blk = sc_ps[:, SQ * h + s0:SQ * h + s0 + 128]
                            nc.vector.tensor_add(blk, blk, mask_tri)
                    attnT = attnp.tile([128, 2 * SQ], FP32)
                    # one exp over both heads' partial-width blocks
                    src = bass.AP(
                        tensor=sc_ps.tensor,
                        offset=sc_ps.offset + s0,
                        ap=[sc_ps.ap[0], [SQ, 2], [1, w]],
                    )
                    dst = bass.AP(
                        tensor=attnT.tensor,
                        offset=attnT.offset + s0,
                        ap=[attnT.ap[0], [SQ, 2], [1, w]],
                    )
                    nc.scalar.activation(dst, src, mybir.ActivationFunctionType.Exp)
                    for h in range(2):
                        mm(
                            av_ps[h][:, s0:SQ],
                            lhsT=v_sb[:, i, h * (DV + 1):(h + 1) * (DV + 1)],
                            rhs=attnT[:, SQ * h + s0:SQ * (h + 1)],
                            start=(i == 0),
                            stop=(i == n_i - 1),
                        )
                # normalize: rows 0..63 / row 64
                for h in range(2):
                    recip = smallp.tile([1, SQ], FP32, tag="recip")
                    nc.vector.reciprocal(recip, av_ps[h][DV:DV + 1, :])
                    rbounce = dramp.tile([SQ], FP32, tag="rbounce")
                    nc.sync.dma_start(out=rbounce, in_=recip)
                    bcast = smallp.tile([DV, SQ], FP32, tag="bcast")
                    nc.gpsimd.dma_start(
                        out=bcast,
                        in_=bass.AP(
                            tensor=rbounce.tensor,
                            offset=rbounce.offset,
                            ap=[[0, DV], [1, SQ]],
                        ),
                    )
                    dst = (
                        outT_full[0:DV, bass.ts(j, SQ)]
                        if h == 0
                        else outT_h1[:, bass.ts(j, SQ)]
                    )
                    nc.vector.tensor_mul(dst, av_ps[h][0:DV, :], bcast)
            # shift head1 rows to partitions 64..127
            nc.gpsimd.dma_start(out=outT_full[DV:128, :], in_=outT_h1)

            # ---- output projection ----
            for t in range(S // 128):
                yst = ystp.tile([128, D], FP32)
                for n in range(D // SQ):
                    yps = ps_work.tile([128, SQ], FP32, tag="ps_work")
                    mm(
                        yps,
                        lhsT=outT_full[:, bass.ts(t, 128)],
                        rhs=wout_sb[:, bass.ts(n, SQ)],
                        start=True,
                        stop=True,
                    )
                    nc.vector.tensor_copy(yst[:, bass.ts(n, SQ)], yps)
                nc.sync.dma_start(
                    out=y[b, 128 * t:128 * (t + 1), :], in_=yst
                )


def shard_inputs(X, W_qkv, W_out):
    """Host-side sharding. Returns per-core input maps."""
    X = np.ascontiguousarray(np.asarray(X, dtype=np.float32))
    W_qkv = np.asarray(W_qkv, dtype=np.float32)
    W_out = np.asarray(W_out, dtype=np.float32)
    xt = np.ascontiguousarray(X.transpose(0, 2, 1))  # [B, D, S]
    scale = np.float32(1.0 / np.sqrt(DV))
    in_maps = []
    for c in range(N_CORES):
        r = slice(E * c, E * (c + 1))
        wq = W_qkv[0 * D:1 * D][r] * scale
        wk = W_qkv[1 * D:2 * D][r]
        wv = W_qkv[2 * D:3 * D][r]
        in_maps.append({
            "xt": xt,
            "wqT": np.ascontiguousarray(wq.T),
            "wkT": np.ascontiguousarray(wk.T),
            "wvT": np.ascontiguousarray(wv.T),
            "woutT": np.ascontiguousarray(W_out[:, r].T),
        })
    return in_maps


def kernel(X, W_qkv, W_out):
    from concourse.bass_utils import run_bass_kernel_spmd

    nc = build_nc()
    in_maps = shard_inputs(X, W_qkv, W_out)
    res = run_bass_kernel_spmd(nc, in_maps, core_ids=list(range(N_CORES)))
    out = np.zeros((B, S, D), dtype=np.float32)
    for r in res.results:
        out += r["y"]
    return out


# revision 21
# speedup vs baseline: 1.5595x; 1.5595x over previous
"""Causal multi-head self-attention on 8 Trainium2 NeuronCores.

Sharding: tensor-parallel over heads. 16 heads / 8 cores = 2 heads per core.
Each core gets the W_qkv rows and W_out columns for its 2 heads, the full
(pre-transposed) X, computes its heads' attention plus its slice of the output
projection, and returns a partial [B, S, D] output. The host sums the 8
partials (the "all-reduce" of the TP output projection).

On-device layout strategy (everything stays transposed until the end):
  - qkvT = W_shard @ X^T computed as matmul(lhsT=W^T tile, rhs=X^T tile)
    -> Q^T/K^T/V^T tiles [dv-part, seq-free]; head0 on partitions 0-63,
    head1 on 64-127.
  - scoresT[kpos, q] = matmul(lhsT=K^T tile, rhs=Q^T tile); the two heads run
    concurrently on the PE array via row-tiling (contraction dv=64 each).
  - causal handling: only q >= kpos tiles/columns are computed (partial-width
    matmuls); the 128-wide diagonal block gets a precomputed triangular
    -1e30 mask added before exp.
  - softmax without max-subtraction (scores ~ N(0,1): exp is safe in fp32);
    exp on the scalar engine reads PSUM directly, one call for both heads.
  - V is re-transposed to natural layout with PE transposes; an extra
    all-ones column is appended so the attn@V matmul also produces the
    softmax denominators in PSUM row 64 for free.
  - normalization: reciprocal of the sums row, partition-broadcast via DMA,
    one tensor-tensor multiply into the out^T buffer.
  - output projection y[s,dm] = matmul(lhsT=outT tile [e=128, s], rhs=W_out^T
    shard) with a single k=128 contraction (head1's outT is shifted to
    partitions 64-127 by an SBUF->SBUF DMA).
"""

import numpy as np

import concourse.bacc as bacc
import concourse.bass as bass
import concourse.mybir as mybir
import concourse.tile as tile

FP32 = mybir.dt.float32

B = 4
S = 2048
D = 1024
H = 16
DV = 64
N_CORES = 8
HEADS_PER_CORE = H // N_CORES          # 2
E = HEADS_PER_CORE * DV                # 128 rows of Q/K/V per core
NEG = -1.0e30

# PE matmul operand dtype: float32 is exact but runs at 4 cycles/row on the
# PE; float32r is the single-pass reduced-precision fp32 mode (1 cycle/row
# when N>=256). Storage is identical (4-byte); the BIR verifier requires the
# whole producer chain of a matmul operand to be typed float32r, so the DRAM
# params and the SBUF tiles feeding matmuls are declared MM_DT.
MM_DT = mybir.dt.float32r

SQ = 512            # q tile width (PSUM bank)
SK = 128            # kpos tile width (contraction)
N_SQ = S // SQ      # 4 q-tiles per (b, h)
N_SK = S // SK      # 16 kpos tiles
N_D = D // 128      # 8 contraction tiles for the projections
VBLK = 2 * (DV + 1)  # 130: [V_h0 | 1 | V_h1 | 1] per kpos tile


def build_nc() -> bass.Bass:
    # Bacc (not plain Bass): its compile() pass splits multi-wait
    # instructions that walrus codegen otherwise rejects ("Too many sync
    # wait commands" — the ISA has one wait slot per instruction).
    nc = bacc.Bacc(None, target_bir_lowering=False)

    xt = nc.declare_dram_parameter("xt", [B, D, S], MM_DT, isOutput=False)
    wqT = nc.declare_dram_parameter("wqT", [D, E], MM_DT, isOutput=False)
    wkT = nc.declare_dram_parameter("wkT", [D, E], MM_DT, isOutput=False)
    wvT = nc.declare_dram_parameter("wvT", [D, E], MM_DT, isOutput=False)
    woutT = nc.declare_dram_parameter("woutT", [E, D], MM_DT, isOutput=False)
    y = nc.declare_dram_parameter("y", [B, S, D], FP32, isOutput=True)

    with tile.TileContext(nc) as tc:
        _build(tc, xt, wqT, wkT, wvT, woutT, y)
    nc.compile()
    return nc


def _build(tc, xt, wqT, wkT, wvT, woutT, y):
    nc = tc.nc

    def mm(out, lhsT, rhs, start, stop):
        nc.tensor.matmul(out, lhsT=lhsT, rhs=rhs, start=start, stop=stop)

    with (
        tc.tile_pool(name="consts", bufs=1) as consts,
        tc.tile_pool(name="xtp", bufs=1) as xtp,
        tc.tile_pool(name="qkp", bufs=2) as qkp,
        tc.tile_pool(name="vtp", bufs=1) as vtp,
        tc.tile_pool(name="vnp", bufs=2) as vnp,
        tc.tile_pool(name="attnp", bufs=3) as attnp,
        tc.tile_pool(name="outp", bufs=2) as outp,
        tc.tile_pool(name="smallp", bufs=2) as smallp,
        tc.tile_pool(name="ystp", bufs=2) as ystp,
        tc.tile_pool(name="dramp", bufs=4, space="DRAM") as dramp,
        tc.tile_pool(name="ps_work", bufs=2, space="PSUM") as ps_work,
        tc.tile_pool(name="ps_scores", bufs=2, space="PSUM") as ps_scores,
        tc.tile_pool(name="ps_av", bufs=2, space="PSUM") as ps_av,
    ):
        # ---- constants ----
        identity = consts.tile([128, 128], FP32)
        nc.gpsimd.memset(identity, 0.0)
        nc.gpsimd.affine_select(
            out=identity, in_=identity,
            compare_op=mybir.AluOpType.not_equal,
            fill=1.0, base=0, pattern=[[-1, 128]], channel_multiplier=1,
        )
        # mask_tri[p, c] = 0 if c >= p else NEG   (valid where q-col >= kpos-row)
        mask_tri = consts.tile([128, 128], FP32)
        nc.gpsimd.memset(mask_tri, 0.0)
        nc.gpsimd.affine_select(
            out=mask_tri, in_=mask_tri,
            compare_op=mybir.AluOpType.is_ge,
            fill=NEG, base=0, pattern=[[1, 128]], channel_multiplier=-1,
        )

        # weights: [D, E] -> [128p, N_D, E] tiles (lhsT layout, contraction on
        # partitions)
        w_sb = {}
        for name, w in (("q", wqT), ("k", wkT), ("v", wvT)):
            t = consts.tile([128, N_D, E], MM_DT, tag=f"w{name}_sb")
            nc.sync.dma_start(out=t, in_=w[:].rearrange("(t p) e -> p t e", p=128))
            w_sb[name] = t
        wout_sb = consts.tile([128, D], MM_DT)
        nc.sync.dma_start(out=wout_sb, in_=woutT[:])
        # fp32 ones source for the fp32r ones-columns of v_sb (memset can't
        # write fp32r, but a casting DVE copy can)
        ones32 = consts.tile([128, 32], FP32)
        nc.gpsimd.memset(ones32, 1.0)

        for b in range(B):
            # ---- load X^T for this batch ----
            # one tile+DMA per 128-row block so each matmul waits on only one
            # DMA semaphore (walrus rejects >4 sync waits per instruction)
            xt_sb = [
                xtp.tile([128, S], MM_DT, tag=f"xt{t}", name=f"xt_sb{t}")
                for t in range(N_D)
            ]
            for t in range(N_D):
                nc.sync.dma_start(
                    out=xt_sb[t], in_=xt[b, 128 * t:128 * (t + 1), :]
                )

            # ---- QKV projections (transposed layout) ----
            qT_sb = qkp.tile([128, S], MM_DT, tag="qT")
            kT_sb = qkp.tile([128, S], MM_DT, tag="kT")
            vT_sb = vtp.tile([128, S], FP32)
            for wname, dst in (("q", qT_sb), ("k", kT_sb), ("v", vT_sb)):
                for j in range(N_SQ):
                    ps = ps_work.tile([128, SQ], FP32, tag="ps_work")
                    for d in range(N_D):
                        mm(
                            ps,
                            lhsT=w_sb[wname][:, d, :],
                            rhs=xt_sb[d][:, bass.ts(j, SQ)],
                            start=(d == 0),
                            stop=(d == N_D - 1),
                        )
                    nc.vector.tensor_copy(dst[:, bass.ts(j, SQ)], ps)

            # ---- V -> natural layout with ones columns ----
            # v_sb block i: [V_h0(64) | 1 | V_h1(64) | 1]
            v_sb = vnp.tile([128, N_SK, VBLK], MM_DT)
            ones_ap = bass.AP(
                tensor=v_sb.tensor,
                offset=v_sb.offset + DV,
                ap=[v_sb.ap[0], [VBLK, N_SK], [DV + 1, 2]],
            )
            nc.vector.tensor_copy(
                ones_ap,
                bass.AP(
                    tensor=ones32.tensor,
                    offset=ones32.offset,
                    ap=[ones32.ap[0], [2, N_SK], [1, 2]],
                ),
            )
            for i in range(N_SK):
                tps = ps_work.tile([128, 128], FP32, tag="ps_work")
                nc.tensor.transpose(tps, vT_sb[:, bass.ts(i, SK)], identity)
                nc.vector.tensor_copy(v_sb[:, i, 0:DV], tps[:, 0:DV])
                nc.vector.tensor_copy(v_sb[:, i, DV + 1:DV + 1 + DV], tps[:, DV:128])

            # ---- attention ----
            outT_full = outp.tile([128, S], MM_DT, tag="outT_full")
            outT_h1 = outp.tile([64, S], MM_DT, tag="outT_h1", bufs=1)
            for j in range(N_SQ):
                av_ps = [
                    ps_av.tile([DV + 1, SQ], FP32, tag="av_ps", name=f"av_ps{h}")
                    for h in range(2)
                ]
                n_i = 4 * j + 4
                for i in range(n_i):
                    s0 = max(0, SK * i - SQ * j)  # first valid col in q block
                    w = SQ - s0
                    sc_ps = ps_scores.tile([128, 2 * SQ], FP32, tag="sc_ps")
                    for h in range(2):
                        mm(
                            sc_ps[:, SQ * h + s0:SQ * (h + 1)],
                            lhsT=kT_sb[DV * h:DV * (h + 1), bass.ts(i, SK)],
                            rhs=qT_sb[DV * h:DV * (h + 1), SQ * j + s0:SQ * (j + 1)],
                            start=True,
                            stop=True,
                        )
                    if i >= 4 * j:  # diagonal tile: triangular mask on 128 cols
                        for h in range(2):
                            blk = sc_ps[:, SQ * h + s0:SQ * h + s0 + 128]
                            nc.vector.tensor_add(blk, blk, mask_tri)
                    attnT = attnp.tile([128, 2 * SQ], MM_DT)
                    # one exp over both heads' partial-width blocks
                    src = bass.AP(
                        tensor=sc_ps.tensor,
                        offset=sc_ps.offset + s0,
                        ap=[sc_ps.ap[0], [SQ, 2], [1, w]],
                    )
                    dst = bass.AP(
                        tensor=attnT.tensor,
                        offset=attnT.offset + s0,
                        ap=[attnT.ap[0], [SQ, 2], [1, w]],
                    )
                    nc.scalar.activation(dst, src, mybir.ActivationFunctionType.Exp)
                    for h in range(2):
                        mm(
                            av_ps[h][:, s0:SQ],
                            lhsT=v_sb[:, i, h * (DV + 1):(h + 1) * (DV + 1)],
                            rhs=attnT[:, SQ * h + s0:SQ * (h + 1)],
                            start=(i == 0),
                            stop=(i == n_i - 1),
                        )
                # normalize: rows 0..63 / row 64
                for h in range(2):
                    recip = smallp.tile([1, SQ], FP32, tag="recip")
                    nc.vector.reciprocal(recip, av_ps[h][DV:DV + 1, :])
                    rbounce = dramp.tile([SQ], FP32, tag="rbounce")
                    nc.sync.dma_start(out=rbounce, in_=recip)
                    bcast = smallp.tile([DV, SQ], FP32, tag="bcast")
                    nc.gpsimd.dma_start(
                        out=bcast,
                        in_=bass.AP(
                            tensor=rbounce.tensor,
                            offset=rbounce.offset,
                            ap=[[0, DV], [1, SQ]],
                        ),
                    )
                    dst = (
                        outT_full[0:DV, bass.ts(j, SQ)]
                        if h == 0
                        else outT_h1[:, bass.ts(j, SQ)]
                    )
                    nc.vector.tensor_mul(dst, av_ps[h][0:DV, :], bcast)
            # shift head1 rows to partitions 64..127
            nc.gpsimd.dma_start(out=outT_full[DV:128, :], in_=outT_h1)

            # ---- output projection ----
            for t in range(S // 128):
                yst = ystp.tile([128, D], FP32)
                for n in range(D // SQ):
                    yps = ps_work.tile([128, SQ], FP32, tag="ps_work")
                    mm(
                        yps,
                        lhsT=outT_full[:, bass.ts(t, 128)],
                        rhs=wout_sb[:, bass.ts(n, SQ)],
                        start=True,
                        stop=True,
                    )
                    nc.vector.tensor_copy(yst[:, bass.ts(n, SQ)], yps)
                nc.sync.dma_start(
                    out=y[b, 128 * t:128 * (t + 1), :], in_=yst
                )


def shard_inputs(X, W_qkv, W_out):
    """Host-side sharding. Returns per-core input maps."""
    X = np.ascontiguousarray(np.asarray(X, dtype=np.float32))
    W_qkv = np.asarray(W_qkv, dtype=np.float32)
    W_out = np.asarray(W_out, dtype=np.float32)
    xt = np.ascontiguousarray(X.transpose(0, 2, 1))  # [B, D, S]
    scale = np.float32(1.0 / np.sqrt(DV))
    in_maps = []
    for c in range(N_CORES):
        r = slice(E * c, E * (c + 1))
        wq = W_qkv[0 * D:1 * D][r] * scale
        wk = W_qkv[1 * D:2 * D][r]
        wv = W_qkv[2 * D:3 * D][r]
        in_maps.append({
            "xt": xt,
            "wqT": np.ascontiguousarray(wq.T),
            "wkT": np.ascontiguousarray(wk.T),
            "wvT": np.ascontiguousarray(wv.T),
            "woutT": np.ascontiguousarray(W_out[:, r].T),
        })
    return in_maps


def kernel(X, W_qkv, W_out):
    from concourse.bass_utils import run_bass_kernel_spmd

    nc = build_nc()
    in_maps = shard_inputs(X, W_qkv, W_out)
    res = run_bass_kernel_spmd(nc, in_maps, core_ids=list(range(N_CORES)))
    out = np.zeros((B, S, D), dtype=np.float32)
    for r in res.results:
        out += r["y"]
    return out


# revision 23
# speedup vs baseline: 1.6747x; 1.0739x over previous
"""Causal multi-head self-attention on 8 Trainium2 NeuronCores.

Sharding: tensor-parallel over heads. 16 heads / 8 cores = 2 heads per core.
Each core gets the W_qkv rows and W_out columns for its 2 heads, the full
(pre-transposed) X, computes its heads' attention plus its slice of the output
projection, and returns a partial [B, S, D] output. The host sums the 8
partials (the "all-reduce" of the TP output projection).

On-device layout strategy (everything stays transposed until the end):
  - qkvT = W_shard @ X^T computed as matmul(lhsT=W^T tile, rhs=X^T tile)
    -> Q^T/K^T/V^T tiles [dv-part, seq-free]; head0 on partitions 0-63,
    head1 on 64-127.
  - scoresT[kpos, q] = matmul(lhsT=K^T tile, rhs=Q^T tile); the two heads run
    concurrently on the PE array via row-tiling (contraction dv=64 each).
  - causal handling: only q >= kpos tiles/columns are computed (partial-width
    matmuls); the 128-wide diagonal block gets a precomputed triangular
    -1e30 mask added before exp.
  - softmax without max-subtraction (scores ~ N(0,1): exp is safe in fp32);
    exp on the scalar engine reads PSUM directly, one call for both heads.
  - V is re-transposed to natural layout with PE transposes; an extra
    all-ones column is appended so the attn@V matmul also produces the
    softmax denominators in PSUM row 64 for free.
  - normalization: reciprocal of the sums row, partition-broadcast via DMA,
    one tensor-tensor multiply into the out^T buffer.
  - output projection y[s,dm] = matmul(lhsT=outT tile [e=128, s], rhs=W_out^T
    shard) with a single k=128 contraction (head1's outT is shifted to
    partitions 64-127 by an SBUF->SBUF DMA).
"""

import numpy as np

import concourse.bacc as bacc
import concourse.bass as bass
import concourse.mybir as mybir
import concourse.tile as tile

FP32 = mybir.dt.float32

B = 4
S = 2048
D = 1024
H = 16
DV = 64
N_CORES = 8
HEADS_PER_CORE = H // N_CORES          # 2
E = HEADS_PER_CORE * DV                # 128 rows of Q/K/V per core
NEG = -1.0e30

# PE matmul operand dtype: float32 is exact but runs at 4 cycles/row on the
# PE; float32r is the single-pass reduced-precision fp32 mode (1 cycle/row
# when N>=256). Storage is identical (4-byte); the BIR verifier requires the
# whole producer chain of a matmul operand to be typed float32r, so the DRAM
# params and the SBUF tiles feeding matmuls are declared MM_DT.
MM_DT = mybir.dt.float32r

SQ = 512            # q tile width (PSUM bank)
SK = 128            # kpos tile width (contraction)
N_SQ = S // SQ      # 4 q-tiles per (b, h)
N_SK = S // SK      # 16 kpos tiles
N_D = D // 128      # 8 contraction tiles for the projections
VBLK = 2 * (DV + 1)  # 130: [V_h0 | 1 | V_h1 | 1] per kpos tile


def build_nc() -> bass.Bass:
    # Bacc (not plain Bass): its compile() pass splits multi-wait
    # instructions that walrus codegen otherwise rejects ("Too many sync
    # wait commands" — the ISA has one wait slot per instruction).
    nc = bacc.Bacc(None, target_bir_lowering=False)

    xt = nc.declare_dram_parameter("xt", [B, D, S], MM_DT, isOutput=False)
    wqT = nc.declare_dram_parameter("wqT", [D, E], MM_DT, isOutput=False)
    wkT = nc.declare_dram_parameter("wkT", [D, E], MM_DT, isOutput=False)
    wvT = nc.declare_dram_parameter("wvT", [D, E], MM_DT, isOutput=False)
    woutT = nc.declare_dram_parameter("woutT", [E, D], MM_DT, isOutput=False)
    y = nc.declare_dram_parameter("y", [B, S, D], FP32, isOutput=True)

    with tile.TileContext(nc) as tc:
        _build(tc, xt, wqT, wkT, wvT, woutT, y)
    nc.compile()
    return nc


def _build(tc, xt, wqT, wkT, wvT, woutT, y):
    nc = tc.nc

    def mm(out, lhsT, rhs, start, stop):
        nc.tensor.matmul(out, lhsT=lhsT, rhs=rhs, start=start, stop=stop)

    with (
        tc.tile_pool(name="consts", bufs=1) as consts,
        tc.tile_pool(name="xtp", bufs=1) as xtp,
        tc.tile_pool(name="qkp", bufs=2) as qkp,
        tc.tile_pool(name="vtp", bufs=1) as vtp,
        tc.tile_pool(name="vnp", bufs=2) as vnp,
        tc.tile_pool(name="attnp", bufs=3) as attnp,
        tc.tile_pool(name="outp", bufs=2) as outp,
        tc.tile_pool(name="smallp", bufs=2) as smallp,
        tc.tile_pool(name="ystp", bufs=3) as ystp,
        tc.tile_pool(name="dramp", bufs=4, space="DRAM") as dramp,
        tc.tile_pool(name="ps_work", bufs=2, space="PSUM") as ps_work,
        tc.tile_pool(name="ps_scores", bufs=2, space="PSUM") as ps_scores,
        tc.tile_pool(name="ps_av", bufs=2, space="PSUM") as ps_av,
    ):
        # ---- constants ----
        identity = consts.tile([128, 128], FP32)
        nc.gpsimd.memset(identity, 0.0)
        nc.gpsimd.affine_select(
            out=identity, in_=identity,
            compare_op=mybir.AluOpType.not_equal,
            fill=1.0, base=0, pattern=[[-1, 128]], channel_multiplier=1,
        )
        # mask_tri[p, c] = 0 if c >= p else NEG   (valid where q-col >= kpos-row)
        mask_tri = consts.tile([128, 128], FP32)
        nc.gpsimd.memset(mask_tri, 0.0)
        nc.gpsimd.affine_select(
            out=mask_tri, in_=mask_tri,
            compare_op=mybir.AluOpType.is_ge,
            fill=NEG, base=0, pattern=[[1, 128]], channel_multiplier=-1,
        )

        # weights: [D, E] -> [128p, N_D, E] tiles (lhsT layout, contraction on
        # partitions)
        w_sb = {}
        for name, w in (("q", wqT), ("k", wkT), ("v", wvT)):
            t = consts.tile([128, N_D, E], MM_DT, tag=f"w{name}_sb")
            nc.sync.dma_start(out=t, in_=w[:].rearrange("(t p) e -> p t e", p=128))
            w_sb[name] = t
        wout_sb = consts.tile([128, D], MM_DT)
        nc.sync.dma_start(out=wout_sb, in_=woutT[:])
        # fp32 ones source for the fp32r ones-columns of v_sb (memset can't
        # write fp32r, but a casting DVE copy can)
        ones32 = consts.tile([128, 32], FP32)
        nc.gpsimd.memset(ones32, 1.0)

        for b in range(B):
            # ---- load X^T for this batch ----
            # one tile+DMA per 128-row block so each matmul waits on only one
            # DMA semaphore (walrus rejects >4 sync waits per instruction)
            xt_sb = [
                xtp.tile([128, S], MM_DT, tag=f"xt{t}", name=f"xt_sb{t}")
                for t in range(N_D)
            ]
            for t in range(N_D):
                nc.sync.dma_start(
                    out=xt_sb[t], in_=xt[b, 128 * t:128 * (t + 1), :]
                )

            # ---- QKV projections (transposed layout) ----
            scope_qkv = nc.named_scope(f"qkv{b}"); scope_qkv.__enter__()
            qT_sb = qkp.tile([128, S], MM_DT, tag="qT")
            kT_sb = qkp.tile([128, S], MM_DT, tag="kT")
            vT_sb = vtp.tile([128, S], FP32)
            for wname, dst in (("q", qT_sb), ("k", kT_sb), ("v", vT_sb)):
                for j in range(N_SQ):
                    ps = ps_work.tile([128, SQ], FP32, tag="ps_work")
                    for d in range(N_D):
                        mm(
                            ps,
                            lhsT=w_sb[wname][:, d, :],
                            rhs=xt_sb[d][:, bass.ts(j, SQ)],
                            start=(d == 0),
                            stop=(d == N_D - 1),
                        )
                    nc.vector.tensor_copy(dst[:, bass.ts(j, SQ)], ps)

            # ---- V -> natural layout with ones columns ----
            # v_sb block i: [V_h0(64) | 1 | V_h1(64) | 1]
            v_sb = vnp.tile([128, N_SK, VBLK], MM_DT)
            ones_ap = bass.AP(
                tensor=v_sb.tensor,
                offset=v_sb.offset + DV,
                ap=[v_sb.ap[0], [VBLK, N_SK], [DV + 1, 2]],
            )
            nc.vector.tensor_copy(
                ones_ap,
                bass.AP(
                    tensor=ones32.tensor,
                    offset=ones32.offset,
                    ap=[ones32.ap[0], [2, N_SK], [1, 2]],
                ),
            )
            for i in range(N_SK):
                tps = ps_work.tile([128, 128], FP32, tag="ps_work")
                nc.tensor.transpose(tps, vT_sb[:, bass.ts(i, SK)], identity)
                vdst = bass.AP(
                    tensor=v_sb.tensor,
                    offset=v_sb.offset + i * VBLK,
                    ap=[v_sb.ap[0], [DV + 1, 2], [1, DV]],
                )
                vsrc = bass.AP(
                    tensor=tps.tensor,
                    offset=tps.offset,
                    ap=[tps.ap[0], [DV, 2], [1, DV]],
                )
                nc.vector.tensor_copy(vdst, vsrc)

            scope_qkv.__exit__(None, None, None)

            # ---- attention ----
            scope_att = nc.named_scope(f"attn{b}"); scope_att.__enter__()
            outT_full = outp.tile([128, S], MM_DT, tag="outT_full")
            outT_h1 = outp.tile([64, S], MM_DT, tag="outT_h1", bufs=1)
            for j in range(N_SQ):
                av_ps = [
                    ps_av.tile([DV + 1, SQ], FP32, tag="av_ps", name=f"av_ps{h}")
                    for h in range(2)
                ]
                n_i = 4 * j + 4
                for i in range(n_i):
                    s0 = max(0, SK * i - SQ * j)  # first valid col in q block
                    w = SQ - s0
                    sc_ps = ps_scores.tile([128, 2 * SQ], FP32, tag="sc_ps")
                    for h in range(2):
                        mm(
                            sc_ps[:, SQ * h + s0:SQ * (h + 1)],
                            lhsT=kT_sb[DV * h:DV * (h + 1), bass.ts(i, SK)],
                            rhs=qT_sb[DV * h:DV * (h + 1), SQ * j + s0:SQ * (j + 1)],
                            start=True,
                            stop=True,
                        )
                    if i >= 4 * j:  # diagonal tile: triangular mask on 128 cols
                        blk = bass.AP(
                            tensor=sc_ps.tensor,
                            offset=sc_ps.offset + s0,
                            ap=[sc_ps.ap[0], [SQ, 2], [1, 128]],
                        )
                        mask2 = bass.AP(
                            tensor=mask_tri.tensor,
                            offset=mask_tri.offset,
                            ap=[mask_tri.ap[0], [0, 2], [1, 128]],
                        )
                        nc.vector.tensor_add(blk, blk, mask2)
                    attnT = attnp.tile([128, 2 * SQ], MM_DT)
                    # one exp over both heads' partial-width blocks
                    src = bass.AP(
                        tensor=sc_ps.tensor,
                        offset=sc_ps.offset + s0,
                        ap=[sc_ps.ap[0], [SQ, 2], [1, w]],
                    )
                    dst = bass.AP(
                        tensor=attnT.tensor,
                        offset=attnT.offset + s0,
                        ap=[attnT.ap[0], [SQ, 2], [1, w]],
                    )
                    nc.scalar.activation(dst, src, mybir.ActivationFunctionType.Exp)
                    for h in range(2):
                        mm(
                            av_ps[h][:, s0:SQ],
                            lhsT=v_sb[:, i, h * (DV + 1):(h + 1) * (DV + 1)],
                            rhs=attnT[:, SQ * h + s0:SQ * (h + 1)],
                            start=(i == 0),
                            stop=(i == n_i - 1),
                        )
                # normalize: rows 0..63 / row 64
                for h in range(2):
                    recip = smallp.tile([1, SQ], FP32, tag="recip")
                    nc.vector.reciprocal(recip, av_ps[h][DV:DV + 1, :])
                    rbounce = dramp.tile([SQ], FP32, tag="rbounce")
                    nc.sync.dma_start(out=rbounce, in_=recip)
                    bcast = smallp.tile([DV, SQ], FP32, tag="bcast")
                    nc.gpsimd.dma_start(
                        out=bcast,
                        in_=bass.AP(
                            tensor=rbounce.tensor,
                            offset=rbounce.offset,
                            ap=[[0, DV], [1, SQ]],
                        ),
                    )
                    dst = (
                        outT_full[0:DV, bass.ts(j, SQ)]
                        if h == 0
                        else outT_h1[:, bass.ts(j, SQ)]
                    )
                    nc.vector.tensor_mul(dst, av_ps[h][0:DV, :], bcast)
            # shift head1 rows to partitions 64..127
            nc.gpsimd.dma_start(out=outT_full[DV:128, :], in_=outT_h1)

            scope_att.__exit__(None, None, None)

            # ---- output projection ----
            scope_y = nc.named_scope(f"yproj{b}"); scope_y.__enter__()
            for t in range(S // 128):
                yst = ystp.tile([128, D], FP32)
                for n in range(D // SQ):
                    yps = ps_work.tile([128, SQ], FP32, tag="ps_work")
                    mm(
                        yps,
                        lhsT=outT_full[:, bass.ts(t, 128)],
                        rhs=wout_sb[:, bass.ts(n, SQ)],
                        start=True,
                        stop=True,
                    )
                    nc.scalar.copy(yst[:, bass.ts(n, SQ)], yps)
                nc.sync.dma_start(
                    out=y[b, 128 * t:128 * (t + 1), :], in_=yst
                )
            scope_y.__exit__(None, None, None)


def shard_inputs(X, W_qkv, W_out):
    """Host-side sharding. Returns per-core input maps."""
    X = np.ascontiguousarray(np.asarray(X, dtype=np.float32))
    W_qkv = np.asarray(W_qkv, dtype=np.float32)
    W_out = np.asarray(W_out, dtype=np.float32)
    xt = np.ascontiguousarray(X.transpose(0, 2, 1))  # [B, D, S]
    scale = np.float32(1.0 / np.sqrt(DV))
    in_maps = []
    for c in range(N_CORES):
        r = slice(E * c, E * (c + 1))
        wq = W_qkv[0 * D:1 * D][r] * scale
        wk = W_qkv[1 * D:2 * D][r]
        wv = W_qkv[2 * D:3 * D][r]
        in_maps.append({
            "xt": xt,
            "wqT": np.ascontiguousarray(wq.T),
            "wkT": np.ascontiguousarray(wk.T),
            "wvT": np.ascontiguousarray(wv.T),
            "woutT": np.ascontiguousarray(W_out[:, r].T),
        })
    return in_maps


def kernel(X, W_qkv, W_out):
    from concourse.bass_utils import run_bass_kernel_spmd

    nc = build_nc()
    in_maps = shard_inputs(X, W_qkv, W_out)
    res = run_bass_kernel_spmd(nc, in_maps, core_ids=list(range(N_CORES)))
    out = np.zeros((B, S, D), dtype=np.float32)
    for r in res.results:
        out += r["y"]
    return out


# revision 27
# speedup vs baseline: 2.4906x; 1.4872x over previous
"""Causal multi-head self-attention on 8 Trainium2 NeuronCores.

Sharding: tensor-parallel over heads. 16 heads / 8 cores = 2 heads per core.
Each core gets the W_qkv rows and W_out columns for its 2 heads, the full
(pre-transposed) X, computes its heads' attention plus its slice of the output
projection, and returns a partial [B, S, D] output. The host sums the 8
partials (the "all-reduce" of the TP output projection).

On-device layout strategy (everything stays transposed until the end):
  - qkvT = W_shard @ X^T computed as matmul(lhsT=W^T tile, rhs=X^T tile)
    -> Q^T/K^T/V^T tiles [dv-part, seq-free]; head0 on partitions 0-63,
    head1 on 64-127.
  - scoresT[kpos, q] = matmul(lhsT=K^T tile, rhs=Q^T tile); the two heads run
    concurrently on the PE array via row-tiling (contraction dv=64 each).
  - causal handling: only q >= kpos tiles/columns are computed (partial-width
    matmuls); the 128-wide diagonal block gets a precomputed triangular
    -1e30 mask added before exp.
  - softmax without max-subtraction (scores ~ N(0,1): exp is safe in fp32);
    exp on the scalar engine reads PSUM directly, one call for both heads.
  - V is re-transposed to natural layout with PE transposes; an extra
    all-ones column is appended so the attn@V matmul also produces the
    softmax denominators in PSUM row 64 for free.
  - normalization: reciprocal of the sums row, partition-broadcast via DMA,
    one tensor-tensor multiply into the out^T buffer.
  - output projection y[s,dm] = matmul(lhsT=outT tile [e=128, s], rhs=W_out^T
    shard) with a single k=128 contraction (head1's outT is shifted to
    partitions 64-127 by an SBUF->SBUF DMA).
"""

import numpy as np

import concourse.bacc as bacc
import concourse.bass as bass
import concourse.mybir as mybir
import concourse.tile as tile

FP32 = mybir.dt.float32

B = 4
S = 2048
D = 1024
H = 16
DV = 64
N_CORES = 8
HEADS_PER_CORE = H // N_CORES          # 2
E = HEADS_PER_CORE * DV                # 128 rows of Q/K/V per core
NEG = -1.0e30

# PE matmul operand dtype. Measured on HW (256x [128,128]x[128,512] MMs):
#   float32:  933 ns/MM, relerr 1.5e-7 (4 cycles/row + unhidden weight load)
#   float32r: 352 ns/MM, relerr 1.6e-4 (1 cycle/row, 4-byte LDW tax ~140ns)
#   float16:  284 ns/MM, relerr 3.1e-4 (1 cycle/row, fast weight load)
# fp16 also halves SBUF footprint and X DMA and doubles DVE copy rates.
# Values are all well inside fp16 range here (|attn| <= ~400).
MM_DT = mybir.dt.float16

SQ = 512            # q tile width (PSUM bank)
SK = 128            # kpos tile width (contraction)
N_SQ = S // SQ      # 4 q-tiles per (b, h)
N_SK = S // SK      # 16 kpos tiles
N_D = D // 128      # 8 contraction tiles for the projections
VBLK = 2 * (DV + 1)  # 130: [V_h0 | 1 | V_h1 | 1] per kpos tile


def build_nc() -> bass.Bass:
    # Bacc (not plain Bass): its compile() pass splits multi-wait
    # instructions that walrus codegen otherwise rejects ("Too many sync
    # wait commands" — the ISA has one wait slot per instruction).
    nc = bacc.Bacc(None, target_bir_lowering=False)

    xt = nc.declare_dram_parameter("xt", [B, D, S], MM_DT, isOutput=False)
    wqT = nc.declare_dram_parameter("wqT", [D, E], MM_DT, isOutput=False)
    wkT = nc.declare_dram_parameter("wkT", [D, E], MM_DT, isOutput=False)
    wvT = nc.declare_dram_parameter("wvT", [D, E], MM_DT, isOutput=False)
    woutT = nc.declare_dram_parameter("woutT", [E, D], MM_DT, isOutput=False)
    y = nc.declare_dram_parameter("y", [B, S, D], FP32, isOutput=True)

    with tile.TileContext(nc) as tc:
        _build(tc, xt, wqT, wkT, wvT, woutT, y)
    nc.compile()
    return nc


def _build(tc, xt, wqT, wkT, wvT, woutT, y):
    nc = tc.nc

    def mm(out, lhsT, rhs, start, stop):
        nc.tensor.matmul(out, lhsT=lhsT, rhs=rhs, start=start, stop=stop)

    with (
        tc.tile_pool(name="consts", bufs=1) as consts,
        tc.tile_pool(name="xtp", bufs=1) as xtp,
        tc.tile_pool(name="qkp", bufs=2) as qkp,
        tc.tile_pool(name="vtp", bufs=1) as vtp,
        tc.tile_pool(name="vnp", bufs=2) as vnp,
        tc.tile_pool(name="attnp", bufs=3) as attnp,
        tc.tile_pool(name="outp", bufs=2) as outp,
        tc.tile_pool(name="smallp", bufs=2) as smallp,
        tc.tile_pool(name="ystp", bufs=3) as ystp,
        tc.tile_pool(name="dramp", bufs=4, space="DRAM") as dramp,
        tc.tile_pool(name="ps_work", bufs=2, space="PSUM") as ps_work,
        tc.tile_pool(name="ps_scores", bufs=2, space="PSUM") as ps_scores,
        tc.tile_pool(name="ps_av", bufs=2, space="PSUM") as ps_av,
    ):
        # ---- constants ----
        identity = consts.tile([128, 128], FP32)
        nc.gpsimd.memset(identity, 0.0)
        nc.gpsimd.affine_select(
            out=identity, in_=identity,
            compare_op=mybir.AluOpType.not_equal,
            fill=1.0, base=0, pattern=[[-1, 128]], channel_multiplier=1,
        )
        # mask_tri01[p, c] = 1 if c >= p else 0 (valid where q-col >= kpos-row);
        # multiplied into attnT after the exp, keeping the PE->ACT psum chain
        # free of DVE ops
        mask_tri01 = consts.tile([128, 128], MM_DT)
        nc.gpsimd.memset(mask_tri01, 1.0)
        nc.gpsimd.affine_select(
            out=mask_tri01, in_=mask_tri01,
            compare_op=mybir.AluOpType.is_ge,
            fill=0.0, base=0, pattern=[[1, 128]], channel_multiplier=-1,
        )

        # weights: [D, E] -> [128p, N_D, E] tiles (lhsT layout, contraction on
        # partitions)
        w_sb = {}
        for name, w in (("q", wqT), ("k", wkT), ("v", wvT)):
            t = consts.tile([128, N_D, E], MM_DT, tag=f"w{name}_sb")
            nc.sync.dma_start(out=t, in_=w[:].rearrange("(t p) e -> p t e", p=128))
            w_sb[name] = t
        wout_sb = consts.tile([128, D], MM_DT)
        nc.sync.dma_start(out=wout_sb, in_=woutT[:])
        # fp32 ones source for the fp32r ones-columns of v_sb (memset can't
        # write fp32r, but a casting DVE copy can)
        ones32 = consts.tile([128, 32], FP32)
        nc.gpsimd.memset(ones32, 1.0)

        for b in range(B):
            # ---- load X^T for this batch ----
            # one tile+DMA per 128-row block so each matmul waits on only one
            # DMA semaphore (walrus rejects >4 sync waits per instruction)
            xt_sb = [
                xtp.tile([128, S], MM_DT, tag=f"xt{t}", name=f"xt_sb{t}")
                for t in range(N_D)
            ]
            for t in range(N_D):
                nc.sync.dma_start(
                    out=xt_sb[t], in_=xt[b, 128 * t:128 * (t + 1), :]
                )

            # ---- QKV projections (transposed layout) ----
            scope_qkv = nc.named_scope(f"qkv{b}"); scope_qkv.__enter__()
            qT_sb = qkp.tile([128, S], MM_DT, tag="qT")
            kT_sb = qkp.tile([128, S], MM_DT, tag="kT")
            vT_sb = vtp.tile([128, S], FP32)
            for wname, dst in (("q", qT_sb), ("k", kT_sb), ("v", vT_sb)):
                for j in range(N_SQ):
                    ps = ps_work.tile([128, SQ], FP32, tag="ps_work")
                    for d in range(N_D):
                        mm(
                            ps,
                            lhsT=w_sb[wname][:, d, :],
                            rhs=xt_sb[d][:, bass.ts(j, SQ)],
                            start=(d == 0),
                            stop=(d == N_D - 1),
                        )
                    nc.vector.tensor_copy(dst[:, bass.ts(j, SQ)], ps)

            # ---- V -> natural layout with ones columns ----
            # v_sb block i: [V_h0(64) | 1 | V_h1(64) | 1]
            v_sb = vnp.tile([128, N_SK, VBLK], MM_DT)
            ones_ap = bass.AP(
                tensor=v_sb.tensor,
                offset=v_sb.offset + DV,
                ap=[v_sb.ap[0], [VBLK, N_SK], [DV + 1, 2]],
            )
            nc.vector.tensor_copy(
                ones_ap,
                bass.AP(
                    tensor=ones32.tensor,
                    offset=ones32.offset,
                    ap=[ones32.ap[0], [2, N_SK], [1, 2]],
                ),
            )
            for i in range(N_SK):
                tps = ps_work.tile([128, 128], FP32, tag="ps_work")
                nc.tensor.transpose(tps, vT_sb[:, bass.ts(i, SK)], identity)
                vdst = bass.AP(
                    tensor=v_sb.tensor,
                    offset=v_sb.offset + i * VBLK,
                    ap=[v_sb.ap[0], [DV + 1, 2], [1, DV]],
                )
                vsrc = bass.AP(
                    tensor=tps.tensor,
                    offset=tps.offset,
                    ap=[tps.ap[0], [DV, 2], [1, DV]],
                )
                nc.vector.tensor_copy(vdst, vsrc)

            scope_qkv.__exit__(None, None, None)

            # ---- attention ----
            scope_att = nc.named_scope(f"attn{b}"); scope_att.__enter__()
            outT_full = outp.tile([128, S], MM_DT, tag="outT_full")
            outT_h1 = outp.tile([64, S], MM_DT, tag="outT_h1", bufs=1)
            for j in range(N_SQ):
                av_ps = [
                    ps_av.tile([DV + 1, SQ], FP32, tag="av_ps", name=f"av_ps{h}")
                    for h in range(2)
                ]
                n_i = 4 * j + 4
                for i in range(n_i):
                    s0 = max(0, SK * i - SQ * j)  # first valid col in q block
                    w = SQ - s0
                    sc_ps = ps_scores.tile([128, 2 * SQ], FP32, tag="sc_ps")
                    for h in range(2):
                        mm(
                            sc_ps[:, SQ * h + s0:SQ * (h + 1)],
                            lhsT=kT_sb[DV * h:DV * (h + 1), bass.ts(i, SK)],
                            rhs=qT_sb[DV * h:DV * (h + 1), SQ * j + s0:SQ * (j + 1)],
                            start=True,
                            stop=True,
                        )
                    attnT = attnp.tile([128, 2 * SQ], MM_DT)
                    # one exp over both heads' partial-width blocks
                    src = bass.AP(
                        tensor=sc_ps.tensor,
                        offset=sc_ps.offset + s0,
                        ap=[sc_ps.ap[0], [SQ, 2], [1, w]],
                    )
                    dst = bass.AP(
                        tensor=attnT.tensor,
                        offset=attnT.offset + s0,
                        ap=[attnT.ap[0], [SQ, 2], [1, w]],
                    )
                    nc.scalar.activation(dst, src, mybir.ActivationFunctionType.Exp)
                    if i >= 4 * j:  # diagonal: zero the upper-triangle entries
                        blk = bass.AP(
                            tensor=attnT.tensor,
                            offset=attnT.offset + s0,
                            ap=[attnT.ap[0], [SQ, 2], [1, 128]],
                        )
                        mask2 = bass.AP(
                            tensor=mask_tri01.tensor,
                            offset=mask_tri01.offset,
                            ap=[mask_tri01.ap[0], [0, 2], [1, 128]],
                        )
                        nc.vector.tensor_mul(blk, blk, mask2)
                    for h in range(2):
                        mm(
                            av_ps[h][:, s0:SQ],
                            lhsT=v_sb[:, i, h * (DV + 1):(h + 1) * (DV + 1)],
                            rhs=attnT[:, SQ * h + s0:SQ * (h + 1)],
                            start=(i == 0),
                            stop=(i == n_i - 1),
                        )
                # evacuate the av psums quickly (a DVE copy), then normalize
                # from the staging copy so the slow recip->DRAM-bounce->bcast
                # chain doesn't hold the PSUM slot for the next j
                for h in range(2):
                    av_st = smallp.tile(
                        [DV + 1, SQ], FP32, tag="av_st", name=f"av_st{h}"
                    )
                    nc.vector.tensor_copy(av_st, av_ps[h])
                    recip = smallp.tile([1, SQ], FP32, tag="recip")
                    nc.vector.reciprocal(recip, av_st[DV:DV + 1, :])
                    rbounce = dramp.tile([SQ], FP32, tag="rbounce")
                    nc.sync.dma_start(out=rbounce, in_=recip)
                    bcast = smallp.tile([DV, SQ], FP32, tag="bcast")
                    nc.gpsimd.dma_start(
                        out=bcast,
                        in_=bass.AP(
                            tensor=rbounce.tensor,
                            offset=rbounce.offset,
                            ap=[[0, DV], [1, SQ]],
                        ),
                    )
                    dst = (
                        outT_full[0:DV, bass.ts(j, SQ)]
                        if h == 0
                        else outT_h1[:, bass.ts(j, SQ)]
                    )
                    nc.vector.tensor_mul(dst, av_st[0:DV, :], bcast)
            # shift head1 rows to partitions 64..127
            nc.gpsimd.dma_start(out=outT_full[DV:128, :], in_=outT_h1)

            scope_att.__exit__(None, None, None)

            # ---- output projection ----
            scope_y = nc.named_scope(f"yproj{b}"); scope_y.__enter__()
            for t in range(S // 128):
                yst = ystp.tile([128, D], FP32)
                for n in range(D // SQ):
                    yps = ps_scores.tile([128, SQ], FP32, tag="sc_ps")
                    mm(
                        yps,
                        lhsT=outT_full[:, bass.ts(t, 128)],
                        rhs=wout_sb[:, bass.ts(n, SQ)],
                        start=True,
                        stop=True,
                    )
                    nc.scalar.copy(yst[:, bass.ts(n, SQ)], yps)
                nc.sync.dma_start(
                    out=y[b, 128 * t:128 * (t + 1), :], in_=yst
                )
            scope_y.__exit__(None, None, None)


def shard_inputs(X, W_qkv, W_out):
    """Host-side sharding. Returns per-core input maps."""
    X = np.ascontiguousarray(np.asarray(X, dtype=np.float32))
    W_qkv = np.asarray(W_qkv, dtype=np.float32)
    W_out = np.asarray(W_out, dtype=np.float32)
    np_mm = mybir.dt.np(MM_DT)
    xt = np.ascontiguousarray(X.transpose(0, 2, 1)).astype(np_mm)  # [B, D, S]
    scale = np.float32(1.0 / np.sqrt(DV))
    in_maps = []
    for c in range(N_CORES):
        r = slice(E * c, E * (c + 1))
        wq = W_qkv[0 * D:1 * D][r] * scale
        wk = W_qkv[1 * D:2 * D][r]
        wv = W_qkv[2 * D:3 * D][r]
        in_maps.append({
            "xt": xt,
            "wqT": np.ascontiguousarray(wq.T).astype(np_mm),
            "wkT": np.ascontiguousarray(wk.T).astype(np_mm),
            "wvT": np.ascontiguousarray(wv.T).astype(np_mm),
            "woutT": np.ascontiguousarray(W_out[:, r].T).astype(np_mm),
        })
    return in_maps


def kernel(X, W_qkv, W_out):
    from concourse.bass_utils import run_bass_kernel_spmd

    nc = build_nc()
    in_maps = shard_inputs(X, W_qkv, W_out)
    res = run_bass_kernel_spmd(nc, in_maps, core_ids=list(range(N_CORES)))
    out = np.zeros((B, S, D), dtype=np.float32)
    for r in res.results:
        out += r["y"]
    return out
